# revision 17
# baseline (speedup 1.0000x reference)
"""Trainium2 Bass kernel for prefix-attention block (B=8,T=1024,C=1024,H=16,Tp=64).

Strategy: data-parallel over batch B across 8 NeuronCores (one batch element
per core, no collectives). Single fused software-pipelined schedule, tuned so
the TensorE never stalls (keeps the PE p-state at full clock):

  - PE stream = projections (as filler) + scores + AV + outproj ONLY. The
    softmax-normalization combine is entirely off the PE: numerators are
    evacuated to SBUF (tA/tB), reciprocals on DVE, per-query broadcast via
    gpsimd casting DMAs, multiply/add on DVE+GpSimd.
  - Causal masks are folded into the score PSUM via small -300-triangle
    matmuls with an identity moving operand (accumulated with start=False),
    so ACT exp feeds AV directly with no mask op in between.
  - ACT engine runs exps (and two tiny sum-row copies per stage) only;
    all PSUM evacuations are on DVE; output stores are bf16 DMAs.
  - Prefix scores for both heads of a pair are packed into one K=128 matmul
    via a materialized block-diagonal stationary (kpbd).
  - Input DMAs split across Sync + ACT + GpSimd queues so the first matmul
    starts as early as possible.
  - Output projection hf1 is split k:(0..6)+(6..8): the k<6 part runs as
    late fillers with a bf16 partial evacuation, leaving a tiny tail.
"""

import numpy as np
import ml_dtypes

B, T, C, H, D, TP = 8, 1024, 1024, 16, 64, 64
NT = T // 128   # 8 token tiles
KC = C // 128   # 8 contraction chunks

_CACHE = {}


def _emit(nc, tc, dram):
    import concourse.bass as bass
    import concourse.mybir as mybir
    from contextlib import ExitStack
    from concourse.tile_rust import add_dep_helper

    BF = mybir.dt.bfloat16
    F32 = mybir.dt.float32
    Exp = mybir.ActivationFunctionType.Exp

    pe_prev = [None]

    def pe_chain(inst):
        if pe_prev[0] is not None:
            add_dep_helper(inst.ins, pe_prev[0].ins, sync=False,
                           reason="forced PE order")
        pe_prev[0] = inst

    with ExitStack() as top:
        top.enter_context(nc.allow_low_precision(
            reason="bf16 compute is intentional; f32 PSUM accumulation"))
        persist = top.enter_context(tc.tile_pool(name="persist", bufs=1))
        ps_gen = top.enter_context(tc.tile_pool(name="ps_gen", bufs=2, space="PSUM"))
        ps_acc = top.enter_context(tc.tile_pool(name="ps_acc", bufs=2, space="PSUM"))

        # ---- persistent SBUF ----
        xT = persist.tile([128, KC * T], BF, tag="xT", name="xT")
        wqk = persist.tile([128, 16 * C], BF, tag="wqk", name="wqk")
        wp = persist.tile([128, 8 * C], BF, tag="wp", name="wp")
        qkT = [persist.tile([128, T], BF, tag=f"qkT{m}", name=f"qkT{m}")
               for m in range(16)]
        vsb = [persist.tile([128, H * 65], BF, tag=f"vsb{t}", name=f"vsb{t}")
               for t in range(NT)]
        kpT = [persist.tile([128, TP], BF, tag=f"kpT{m}", name=f"kpT{m}")
               for m in range(8)]
        kpbd = persist.tile([128, 8 * 128], BF, tag="kpbd", name="kpbd")
        vpsb = persist.tile([128, H * 65], BF, tag="vpsb", name="vpsb")
        identsb = persist.tile([128, 128], BF, tag="identsb", name="identsb")
        trisb = persist.tile([128, 128], BF, tag="trisb", name="trisb")
        ptrisb = persist.tile([64, 128], BF, tag="ptrisb", name="ptrisb")
        yT = [persist.tile([128, T], BF, tag=f"yT{t}", name=f"yT{t}")
              for t in range(NT)]

        # ---- input DMAs: 3 queues (sync / scalar / gpsimd), need order ----
        def w_sliced_dma(eng, dst, dram_t, col0, m_dst):
            # dst[:, (m_dst*KC + k)*128 : +128] = W[k*128:(k+1)*128, col0 : +128]
            dt = dram_t.ap()
            src = bass.AP(tensor=dt.tensor, offset=dt.offset + col0,
                          ap=[[dt.ap[0][0], 128],          # p within chunk
                              [dt.ap[0][0] * 128, KC],     # k
                              [1, 128]])                   # col
            dstv = bass.AP(tensor=dst.tensor,
                           offset=dst.offset + m_dst * KC * 128,
                           ap=[[dst.ap[0][0], 128],
                               [128, KC],
                               [1, 128]])
            eng.dma_start(out=dstv, in_=src)

        # pT/wkp/wv die once the (p,0) fillers and preloop v_halfs are done
        # (stage 8); their space is then reused for part1.
        pearly = ExitStack()
        early_pool = pearly.enter_context(tc.tile_pool(name="pearly", bufs=1))
        wkp = early_pool.tile([128, 8 * C], BF, tag="wkp", name="wkp")
        wv = early_pool.tile([128, KC * C], BF, tag="wv", name="wv")
        pT = early_pool.tile([128, KC * TP], BF, tag="pT", name="pT")
        pwvp = ExitStack()
        wvp = pwvp.enter_context(tc.tile_pool(name="pwvp", bufs=1)).tile(
            [128, KC * C], BF, tag="wvp", name="wvp")

        # sync: preloop needs first — pT, all wkp, wvp, consts, then q/k
        # weights in stage order, wp last (needed only by late fillers)
        nc.sync.dma_start(out=pT.rearrange("p (k t) -> p k t", t=TP),
                          in_=dram["pT"].ap().rearrange("(k p) t -> p k t", p=128))
        for m in range(8):
            w_sliced_dma(nc.sync, wkp, dram["wkp"], m * 128, m)
        nc.sync.dma_start(out=wvp.rearrange("p (k c) -> p k c", c=C),
                          in_=dram["wvp"].ap().rearrange("(k p) c -> p k c", p=128))
        nc.sync.dma_start(out=identsb, in_=dram["ident"].ap())
        nc.sync.dma_start(out=trisb, in_=dram["tri"].ap())
        nc.sync.dma_start(out=ptrisb, in_=dram["ptri"].ap())
        w_sliced_dma(nc.sync, wqk, dram["wqk"], 0, 0)            # q pair 0
        w_sliced_dma(nc.sync, wqk, dram["wqk"], C, 8)            # k pair 0
        w_sliced_dma(nc.sync, wqk, dram["wqk"], 128, 1)          # q pair 1
        w_sliced_dma(nc.sync, wqk, dram["wqk"], C + 128, 9)      # k pair 1
        for m in range(2, 8):
            w_sliced_dma(nc.sync, wqk, dram["wqk"], m * 128, m)
            w_sliced_dma(nc.sync, wqk, dram["wqk"], C + m * 128, 8 + m)
        for m in range(8):
            w_sliced_dma(nc.sync, wp, dram["wp"], m * 128, m)

        # scalar (ACT): x and wv — done long before the first exp
        for k2 in range(4):
            nc.scalar.dma_start(
                out=xT.rearrange("p (k t) -> p k t", t=T)[:, 2 * k2:2 * k2 + 2, :],
                in_=dram["xT"].ap().rearrange("(k p) t -> p k t", p=128)
                [:, 2 * k2:2 * k2 + 2, :])
        for k2 in range(2):
            nc.scalar.dma_start(
                out=wv.rearrange("p (k c) -> p k c", c=C)[:, 4 * k2:4 * k2 + 4, :],
                in_=dram["wv"].ap().rearrange("(k p) c -> p k c", p=128)
                [:, 4 * k2:4 * k2 + 4, :])

        def wqk_s(m, k):
            return wqk[:, (m * KC + k) * 128:(m * KC + k) * 128 + 128]

        def wkp_s(m, k):
            return wkp[:, (m * KC + k) * 128:(m * KC + k) * 128 + 128]

        def wp_s(m, k):
            return wp[:, (m * KC + k) * 128:(m * KC + k) * 128 + 128]

        def xT_s(k, sl=None):
            base = xT[:, k * T:(k + 1) * T]
            return base if sl is None else base[:, sl]

        def wv_s(k):
            return wv[:, k * C:(k + 1) * C]

        def wvp_s(k):
            return wvp[:, k * C:(k + 1) * C]

        def pT_s(k):
            return pT[:, k * TP:(k + 1) * TP]

        # zero kpbd once; kpT_group fills the diagonal 64x64 quadrants
        nc.vector.memset(kpbd, 0.0)

        # ---- projection emitters (granular, for filler interleaving) ----
        def kpT_group(m):
            ps = ps_gen.tile([128, TP], F32, tag="ps_g", name="ps_g")
            for k in range(KC):
                pe_chain(nc.tensor.matmul(ps, wkp_s(m, k), pT_s(k),
                                          start=(k == 0), stop=(k == KC - 1)))
            nc.vector.tensor_copy(kpT[m], ps)
            # block-diagonal stationary for the packed prefix-score matmul
            bd = kpbd[:, m * 128:(m + 1) * 128]
            nc.vector.tensor_copy(bd[0:64, 0:64], kpT[m][0:64, :])
            nc.vector.tensor_copy(bd[64:128, 64:128], kpT[m][64:128, :])

        def qk_half(m, hf, _box):
            ps = ps_gen.tile([128, 512], F32, tag="ps_g", name="ps_g")
            for k in range(KC):
                pe_chain(nc.tensor.matmul(
                    ps, wqk_s(m, k), xT_s(k)[:, hf * 512:(hf + 1) * 512],
                    start=(k == 0), stop=(k == KC - 1)))
            nc.vector.tensor_copy(qkT[m][:, hf * 512:(hf + 1) * 512], ps)

        def v_half(tt, hf, _box):
            ps = ps_gen.tile([128, 512], F32, tag="ps_g", name="ps_g")
            sl = slice(tt * 128, (tt + 1) * 128)
            for k in range(KC):
                pe_chain(nc.tensor.matmul(
                    ps, xT_s(k, sl), wv_s(k)[:, hf * 512:(hf + 1) * 512],
                    start=(k == 0), stop=(k == KC - 1)))
            nc.vector.tensor_copy(
                vsb[tt].rearrange("p (h e) -> p h e", e=65)
                [:, hf * 8:(hf + 1) * 8, 0:64],
                ps.rearrange("p (h e) -> p h e", e=64))
            if hf == 1:
                nc.vector.memset(
                    vsb[tt].rearrange("p (h e) -> p h e", e=65)[:, :, 64:65], 1.0)

        def vpsb_group():
            vpv = vpsb.rearrange("p (h e) -> p h e", e=65)
            for hf in range(2):
                ps = ps_gen.tile([64, 512], F32, tag="ps_g", name="ps_g")
                for k in range(KC):
                    pe_chain(nc.tensor.matmul(
                        ps, pT_s(k), wvp_s(k)[:, hf * 512:(hf + 1) * 512],
                        start=(k == 0), stop=(k == KC - 1)))
                nc.vector.tensor_copy(vpv[0:64, hf * 8:(hf + 1) * 8, 0:64],
                                      ps.rearrange("p (h e) -> p h e", e=64))
                nc.vector.tensor_copy(vpv[64:128, hf * 8:(hf + 1) * 8, 0:64],
                                      ps.rearrange("p (h e) -> p h e", e=64))
            nc.vector.memset(vpv[:, :, 64:65], 1.0)

        # outproj hf0: chunk (m, half): half k-ranges (0:4, 4:8); same PSUM
        def outproj_half(hf, m, half, ps_box):
            if half == 0:
                ps_box[0] = ps_gen.tile([128, 512], F32, tag="ps_g", name="ps_g")
            ps = ps_box[0]
            for k in range(4 * half, 4 * half + 4):
                pe_chain(nc.tensor.matmul(
                    ps, wp_s(m, k), yT[k][:, hf * 512:(hf + 1) * 512],
                    start=(k == 0), stop=(k == KC - 1)))
            if half == 1:
                stg = pstg.tile([128, 512], BF, tag="stg", name="stg")
                nc.vector.tensor_copy(stg, ps)
                nc.sync.dma_start(
                    out=dram["out"].ap()[m * 128:(m + 1) * 128,
                                         hf * 512:(hf + 1) * 512],
                    in_=stg)
                ps_box[0] = None

        # outproj hf1 partials: k 0:6 accumulated, evacuated bf16 to part1[m]
        part1_box = [None]

        def outproj1_part(m, half, ps_box):
            if half == 0:
                ps_box[0] = ps_gen.tile([128, 512], F32, tag="ps_g", name="ps_g")
            ps = ps_box[0]
            for k in range(3 * half, 3 * half + 3):
                pe_chain(nc.tensor.matmul(
                    ps, wp_s(m, k), yT[k][:, 512:1024],
                    start=(k == 0), stop=(k == 5)))
            if half == 1:
                part1 = part1_box[0]
                nc.vector.tensor_copy(part1[:, m * 512:(m + 1) * 512], ps)
                ps_box[0] = None

        # ---- filler queue ----
        fillers = []

        def add_group(key, fn, nargs):
            box = [None]
            fillers.append((key, lambda: fn(*nargs, 0, box)))
            fillers.append((key, lambda: fn(*nargs, 1, box)))

        for p in range(2, 8):
            fillers.append(((p, 0), lambda m=p: qk_half(m, 0, None)))
            fillers.append(((p, 0), lambda m=8 + p: qk_half(m, 0, None)))
        for p in range(8):
            fillers.append(((p, 1), lambda m=p: qk_half(m, 1, None)))
            fillers.append(((p, 1), lambda m=8 + p: qk_half(m, 1, None)))
        for m in range(8):
            add_group("op0", outproj_half, (0, m))
        for m in range(8):
            add_group("op1", outproj1_part, (m,))

        def pull_filler(allow_op0, allow_op1=False):
            while fillers:
                key, f = fillers[0]
                if key == "op0" and not allow_op0:
                    return False
                if key == "op1" and not allow_op1:
                    return False
                fillers.pop(0)
                f()
                return True
            return False

        def drain_until(stage_key):
            while any(k == stage_key for k, _ in fillers):
                key, f = fillers.pop(0)
                f()

        class Stage:
            def __init__(self, p, ir):
                self.p, self.ir = p, ir
                self.i0 = ir * 512
                self.jmax = 4 * (ir + 1)
                self.qt, self.kt = qkT[p], qkT[8 + p]
                self.s_all, self.e_all = {}, {}

            def scores(self, jb):
                c0 = max(0, jb - 4 * self.ir) * 128
                st = ps_gen.tile([128, 1024], F32, tag="ps_g", name="ps_g")
                diag = jb >= 4 * self.ir
                for hh, pb in enumerate((0, 64)):
                    pe_chain(nc.tensor.matmul(
                        st[:, hh * 512 + c0:hh * 512 + 512],
                        self.kt[pb:pb + 64, jb * 128:(jb + 1) * 128],
                        self.qt[pb:pb + 64, self.i0 + c0:self.i0 + 512],
                        start=True, stop=not diag))
                if diag:
                    # fold the causal mask in: -300 strict-lower triangle
                    # (key>query) accumulated onto the diagonal 128x128 block
                    for hh in range(2):
                        pe_chain(nc.tensor.matmul(
                            st[:, hh * 512 + c0:hh * 512 + c0 + 128],
                            trisb, identsb,
                            start=False, stop=True, skip_group_check=True))
                self.s_all[jb] = st

            def exps(self, jb):
                c0 = max(0, jb - 4 * self.ir) * 128
                st = self.s_all.pop(jb)
                et = pexp.tile([128, 1024], BF, tag="et", name="et")
                nc.scalar.activation(
                    et.rearrange("p (g n) -> p g n", g=2)[:, :, c0:512],
                    st.rearrange("p (g n) -> p g n", g=2)[:, :, c0:512],
                    Exp, scale=0.125)
                self.e_all[jb] = et

            def avs(self, jb):
                c0 = max(0, jb - 4 * self.ir) * 128
                et = self.e_all.pop(jb)
                for hh in range(2):
                    h = 2 * self.p + hh
                    pe_chain(nc.tensor.matmul(
                        self.Ats[:, hh * 512 + c0:hh * 512 + 512],
                        vsb[jb][:, h * 65:(h + 1) * 65],
                        et[:, hh * 512 + c0:hh * 512 + 512],
                        start=(jb == 0), stop=(jb == self.jmax - 1),
                        skip_group_check=True))

            def front1(self):
                # prefix scores, both heads in one K=128 matmul (block-diag)
                spt = ps_gen.tile([128, 512], F32, tag="ps_g", name="ps_g")
                pe_chain(nc.tensor.matmul(
                    spt, kpbd[:, self.p * 128:(self.p + 1) * 128],
                    self.qt[:, self.i0:self.i0 + 512],
                    start=True, stop=(self.ir == 1)))
                if self.ir == 0:
                    # prefix causal mask on the first 64 query columns
                    pe_chain(nc.tensor.matmul(
                        spt[:, 0:64], ptrisb, identsb[0:64, 0:64],
                        start=False, stop=True, skip_group_check=True))
                self.scores(0)
                ep = pep.tile([128, 512], BF, tag="ep", name="ep")
                nc.scalar.activation(ep, spt, Exp, scale=0.125)
                self.eps = ep
                self.exps(0)

            def front2(self):
                self.scores(1)
                self.Bts = ps_acc.tile([65, 1024], F32, tag="ps_a", name="ps_a")
                for hh, pb in enumerate((0, 64)):
                    h = 2 * self.p + hh
                    pe_chain(nc.tensor.matmul(
                        self.Bts[:, hh * 512:hh * 512 + 512],
                        vpsb[pb:pb + 64, h * 65:(h + 1) * 65],
                        self.eps[pb:pb + 64, :],
                        start=True, stop=True))
                self.exps(1)

            def front2b(self):
                # evacuate B: sums row to recip scratch (ACT, tiny), data to
                # SBUF bf16 (DVE; gpsimd cannot read PSUM). Frees Bts mid-stage.
                self.rs = prs.tile([33, 1024], F32, tag="rs", name="rs")
                nc.scalar.copy(self.rs[0:1, :], self.Bts[64:65, :])
                self.tB = ptb.tile([64, 1024], BF, tag="tB", name="tB")
                nc.vector.tensor_copy(self.tB, self.Bts[0:64, :])
                self.Ats = ps_acc.tile([65, 1024], F32, tag="ps_a", name="ps_a")

            def main(self, allow_op0, allow_op1=False):
                for jb0 in range(0, self.jmax, 2):
                    for jb in (jb0 + 2, jb0 + 3):
                        if jb < self.jmax:
                            self.scores(jb)
                    for jb in (jb0 + 2, jb0 + 3):
                        if jb < self.jmax:
                            self.exps(jb)
                    pull_filler(allow_op0, allow_op1)
                    for jb in (jb0, jb0 + 1):
                        if jb < self.jmax - 1:
                            self.avs(jb)

            def av_last(self):
                self.avs(self.jmax - 1)

            def evacA(self):
                # A sums -> recip scratch row 32 (ACT, tiny); A data -> SBUF
                # bf16 (DVE). This frees the Ats slot: the only thing the next
                # stage's AV waits on is this one DVE copy.
                nc.scalar.copy(self.rs[32:33, :], self.Ats[64:65, :])
                self.tA = pta.tile([64, 1024], BF, tag="tA", name="tA")
                nc.vector.tensor_copy(self.tA, self.Ats[0:64, :])

            def combine(self):
                # recip both sum rows, broadcast per-query recips across
                # partitions with casting gpsimd DMAs, normalize + add.
                # Entirely off the PE.
                rs = self.rs
                nc.vector.reciprocal_approx_fast(rs, rs)
                bcA = pbc.tile([64, 1024], F32, tag="bcA", name="bcA")
                bcB = pbc.tile([64, 1024], F32, tag="bcB", name="bcB")
                rA, rB = rs[32:33, :], rs[0:1, :]
                srcA = bass.AP(tensor=rA.tensor, offset=rA.offset,
                               ap=[[rA.ap[0][0], 1], [0, 64], list(rA.ap[1])])
                srcB = bass.AP(tensor=rB.tensor, offset=rB.offset,
                               ap=[[rB.ap[0][0], 1], [0, 64], list(rB.ap[1])])
                nc.sync.dma_start(out=bcA, in_=srcA)
                nc.sync.dma_start(out=bcB, in_=srcB)
                uA = pua.tile([64, 1024], BF, tag="uA", name="uA")
                nc.vector.tensor_mul(uA, self.tA, bcA)
                uB = pua.tile([64, 1024], BF, tag="uB", name="uB")
                nc.vector.tensor_mul(uB, self.tB, bcB)
                sl = slice(self.i0, self.i0 + 512)
                nc.gpsimd.tensor_add(yT[self.p][0:64, sl],
                                     uA[:, 0:512], uB[:, 0:512])
                nc.gpsimd.tensor_add(yT[self.p][64:128, sl],
                                     uA[:, 512:1024], uB[:, 512:1024])

        # ---- pre-loop: everything that reads pT/wkp/wv/wvp, so their pools
        # can close before the attention pools open (LIFO order) ----
        box = [None]
        for m in range(8):
            kpT_group(m)
        vpsb_group()
        qk_half(0, 0, box); qk_half(8, 0, box)
        qk_half(1, 0, box); qk_half(9, 0, box)
        for tt in range(8):
            v_half(tt, 0, box); v_half(tt, 1, box)
        pwvp.close()   # frees wvp's 16KB
        pearly.close()  # frees pT/wkp/wv (33KB) for part1 + pools below

        plate = top.enter_context(tc.tile_pool(name="plate", bufs=1))
        part1_box[0] = plate.tile([128, 8 * 512], BF, tag="part1", name="part1")
        pexp = top.enter_context(tc.tile_pool(name="pexp", bufs=5))
        pep = top.enter_context(tc.tile_pool(name="pep", bufs=2))
        ptb = top.enter_context(tc.tile_pool(name="ptb", bufs=2))
        pta = top.enter_context(tc.tile_pool(name="pta", bufs=2))
        pbc = top.enter_context(tc.tile_pool(name="pbc", bufs=1))
        pua = top.enter_context(tc.tile_pool(name="pua", bufs=1))
        prs = top.enter_context(tc.tile_pool(name="prs", bufs=2))
        pstg = top.enter_context(tc.tile_pool(name="pstg", bufs=2))
        # recip scratch rotates through two slots; unused rows 1:32 flow
        # through the reciprocal each stage, pin them to 1.0 once.
        for _ in range(2):
            rs_init = prs.tile([33, 1024], F32, tag="rs", name="rs_init")
            nc.vector.memset(rs_init, 1.0)

        # ---- stage loop ----
        stages = [(p, 0) for p in range(8)] + [(p, 1) for p in range(8)]
        prev = None
        for si, (p, ir) in enumerate(stages):
            drain_until((p, ir))
            st = Stage(p, ir)
            st.front1()
            if prev is not None:
                prev.av_last()
                prev.evacA()
            st.front2()
            st.front2b()
            if prev is not None:
                prev.combine()
            st.main(allow_op0=(ir == 1), allow_op1=(si >= 14))
            prev = st
        prev.av_last()
        prev.evacA()
        prev.combine()

        while pull_filler(True, True):
            pass

        # ---- tail: outproj hf1, k 6:8 + bf16 partial add ----
        for m in range(8):
            ps = ps_gen.tile([128, 512], F32, tag="ps_g", name="ps_g")
            for k in (6, 7):
                pe_chain(nc.tensor.matmul(
                    ps, wp_s(m, k), yT[k][:, 512:1024],
                    start=(k == 6), stop=(k == 7)))
            t2 = pstg.tile([128, 512], BF, tag="stg", name="stg")
            nc.vector.tensor_copy(t2, ps)
            stgf = pstg.tile([128, 512], BF, tag="stg", name="stg")
            nc.vector.tensor_add(stgf, t2,
                                 part1_box[0][:, m * 512:(m + 1) * 512])
            nc.sync.dma_start(
                out=dram["out"].ap()[m * 128:(m + 1) * 128, 512:1024],
                in_=stgf)


def _build():
    if "nc" in _CACHE:
        return _CACHE["nc"]
    import concourse.mybir as mybir
    import concourse.tile as tile
    from concourse import bacc

    BF = mybir.dt.bfloat16
    nc = bacc.Bacc("TRN2", target_bir_lowering=False, debug=False,
                   enable_asserts=False)
    dram = {
        "xT": nc.dram_tensor("xT", [C, T], BF, kind="ExternalInput"),
        "pT": nc.dram_tensor("pT", [C, TP], BF, kind="ExternalInput"),
        "wqk": nc.dram_tensor("wqk", [C, 2 * C], BF, kind="ExternalInput"),
        "wv": nc.dram_tensor("wv", [C, C], BF, kind="ExternalInput"),
        "wkp": nc.dram_tensor("wkp", [C, C], BF, kind="ExternalInput"),
        "wvp": nc.dram_tensor("wvp", [C, C], BF, kind="ExternalInput"),
        "wp": nc.dram_tensor("wp", [C, C], BF, kind="ExternalInput"),
        "ident": nc.dram_tensor("ident", [128, 128], BF, kind="ExternalInput"),
        "tri": nc.dram_tensor("tri", [128, 128], BF, kind="ExternalInput"),
        "ptri": nc.dram_tensor("ptri", [64, 128], BF, kind="ExternalInput"),
        "out": nc.dram_tensor("out", [C, T], BF, kind="ExternalOutput"),
    }
    with tile.TileContext(nc) as tc:
        _emit(nc, tc, dram)
    nc.compile()
    _CACHE["nc"] = nc
    return nc


def _host_consts():
    bf = ml_dtypes.bfloat16
    ident = np.eye(128, dtype=np.float32).astype(bf)
    tri = (np.triu(np.ones((128, 128), np.float32), 1) * -300.0).astype(bf)
    p1 = np.triu(np.ones((64, 64), np.float32), 1) * -300.0
    ptri = np.concatenate([p1, p1], axis=1).astype(bf)   # [64 q, 128 keys]
    return ident, tri, ptri


def _make_in_maps(x, prefix_embd, w_attn, w_prefix, w_proj):
    bf = ml_dtypes.bfloat16
    x = np.asarray(x, np.float32)
    prefix_embd = np.asarray(prefix_embd, np.float32)
    w_attn = np.asarray(w_attn, np.float32)
    w_prefix = np.asarray(w_prefix, np.float32)
    w_proj = np.asarray(w_proj, np.float32)
    ident, tri, ptri = _host_consts()
    wqk = np.ascontiguousarray(w_attn[:, :2 * C]).astype(bf)
    wv = np.ascontiguousarray(w_attn[:, 2 * C:]).astype(bf)
    wkp = np.ascontiguousarray(w_prefix[:, C:2 * C]).astype(bf)
    wvp = np.ascontiguousarray(w_prefix[:, 2 * C:]).astype(bf)
    wp = w_proj.astype(bf)
    in_maps = []
    for i in range(B):
        in_maps.append({
            "xT": np.ascontiguousarray(x[i].T).astype(bf),
            "pT": np.ascontiguousarray(prefix_embd[i].T).astype(bf),
            "wqk": wqk, "wv": wv, "wkp": wkp, "wvp": wvp, "wp": wp,
            "ident": ident, "tri": tri, "ptri": ptri,
        })
    return in_maps


def kernel(x, prefix_embd, w_attn, b_attn, w_prefix, b_prefix, w_proj, b_proj,
           **_ignored):
    nc = _build()
    in_maps = _make_in_maps(x, prefix_embd, w_attn, w_prefix, w_proj)
    from concourse.bass_utils import run_bass_kernel_spmd
    res = run_bass_kernel_spmd(nc, in_maps, core_ids=list(range(B)))
    out = np.stack([res.results[i]["out"].T.astype(np.float32)
                    for i in range(B)])
    return np.ascontiguousarray(out)


# revision 18
# speedup vs baseline: 1.3044x; 1.3044x over previous
"""Trainium2 Bass kernel for prefix-attention block (B=8,T=1024,C=1024,H=16,Tp=64).

Strategy: data-parallel over batch B across 8 NeuronCores (one batch element
per core, no collectives). Single fused software-pipelined schedule, tuned so
the TensorE never stalls (keeps the PE p-state at full clock):

  - PE stream = projections (as filler) + scores + AV + outproj ONLY. The
    softmax-normalization combine is entirely off the PE: numerators are
    evacuated to SBUF (tA/tB), reciprocals on DVE, per-query broadcast via
    gpsimd casting DMAs, multiply/add on DVE+GpSimd.
  - Causal masks are folded into the score PSUM via small -300-triangle
    matmuls with an identity moving operand (accumulated with start=False),
    so ACT exp feeds AV directly with no mask op in between.
  - ACT engine runs exps (and two tiny sum-row copies per stage) only;
    all PSUM evacuations are on DVE; output stores are bf16 DMAs.
  - Prefix scores for both heads of a pair are packed into one K=128 matmul
    via a materialized block-diagonal stationary (kpbd).
  - Input DMAs split across Sync + ACT + GpSimd queues so the first matmul
    starts as early as possible.
  - Output projection hf1 is split k:(0..6)+(6..8): the k<6 part runs as
    late fillers with a bf16 partial evacuation, leaving a tiny tail.
"""

import numpy as np
import ml_dtypes

B, T, C, H, D, TP = 8, 1024, 1024, 16, 64, 64
NT = T // 128   # 8 token tiles
KC = C // 128   # 8 contraction chunks

_CACHE = {}


def _emit(nc, tc, dram):
    import concourse.bass as bass
    import concourse.mybir as mybir
    from contextlib import ExitStack
    from concourse.tile_rust import add_dep_helper

    BF = mybir.dt.bfloat16
    F32 = mybir.dt.float32
    Exp = mybir.ActivationFunctionType.Exp

    pe_prev = [None]

    def pe_chain(inst):
        if pe_prev[0] is not None:
            add_dep_helper(inst.ins, pe_prev[0].ins, sync=False,
                           reason="forced PE order")
        pe_prev[0] = inst

    with ExitStack() as top:
        top.enter_context(nc.allow_low_precision(
            reason="bf16 compute is intentional; f32 PSUM accumulation"))
        persist = top.enter_context(tc.tile_pool(name="persist", bufs=1))
        ps_gen = top.enter_context(tc.tile_pool(name="ps_gen", bufs=2, space="PSUM"))
        ps_acc = top.enter_context(tc.tile_pool(name="ps_acc", bufs=2, space="PSUM"))

        # ---- persistent SBUF ----
        xT = persist.tile([128, KC * T], BF, tag="xT", name="xT")
        wqk = persist.tile([128, 16 * C], BF, tag="wqk", name="wqk")
        wp = persist.tile([128, 8 * C], BF, tag="wp", name="wp")
        qkT = [persist.tile([128, T], BF, tag=f"qkT{m}", name=f"qkT{m}")
               for m in range(16)]
        vsb = [persist.tile([128, H * 65], BF, tag=f"vsb{t}", name=f"vsb{t}")
               for t in range(NT)]
        kpT = [persist.tile([128, TP], BF, tag=f"kpT{m}", name=f"kpT{m}")
               for m in range(8)]
        kpbd = persist.tile([128, 8 * 128], BF, tag="kpbd", name="kpbd")
        vpsb = persist.tile([128, H * 65], BF, tag="vpsb", name="vpsb")
        identsb = persist.tile([128, 128], BF, tag="identsb", name="identsb")
        trisb = persist.tile([128, 128], BF, tag="trisb", name="trisb")
        ptrisb = persist.tile([64, 128], BF, tag="ptrisb", name="ptrisb")
        yT = [persist.tile([128, T], BF, tag=f"yT{t}", name=f"yT{t}")
              for t in range(NT)]

        # ---- input DMAs: 3 queues (sync / scalar / gpsimd), need order ----
        def w_sliced_dma(eng, dst, dram_t, col0, m_dst):
            # dst[:, (m_dst*KC + k)*128 : +128] = W[k*128:(k+1)*128, col0 : +128]
            dt = dram_t.ap()
            src = bass.AP(tensor=dt.tensor, offset=dt.offset + col0,
                          ap=[[dt.ap[0][0], 128],          # p within chunk
                              [dt.ap[0][0] * 128, KC],     # k
                              [1, 128]])                   # col
            dstv = bass.AP(tensor=dst.tensor,
                           offset=dst.offset + m_dst * KC * 128,
                           ap=[[dst.ap[0][0], 128],
                               [128, KC],
                               [1, 128]])
            eng.dma_start(out=dstv, in_=src)

        # pT/wkp/wv die once the (p,0) fillers and preloop v_halfs are done
        # (stage 8); their space is then reused for part1.
        pearly = ExitStack()
        early_pool = pearly.enter_context(tc.tile_pool(name="pearly", bufs=1))
        wkp = early_pool.tile([128, 8 * C], BF, tag="wkp", name="wkp")
        wv = early_pool.tile([128, KC * C], BF, tag="wv", name="wv")
        pT = early_pool.tile([128, KC * TP], BF, tag="pT", name="pT")
        pwvp = ExitStack()
        wvp = pwvp.enter_context(tc.tile_pool(name="pwvp", bufs=1)).tile(
            [128, KC * C], BF, tag="wvp", name="wvp")

        # sync: preloop needs first — pT, all wkp, wvp, consts, then q/k
        # weights in stage order, wp last (needed only by late fillers)
        nc.sync.dma_start(out=pT.rearrange("p (k t) -> p k t", t=TP),
                          in_=dram["pT"].ap().rearrange("(k p) t -> p k t", p=128))
        for m in range(8):
            w_sliced_dma(nc.sync, wkp, dram["wkp"], m * 128, m)
        nc.sync.dma_start(out=wvp.rearrange("p (k c) -> p k c", c=C),
                          in_=dram["wvp"].ap().rearrange("(k p) c -> p k c", p=128))
        nc.sync.dma_start(out=identsb, in_=dram["ident"].ap())
        nc.sync.dma_start(out=trisb, in_=dram["tri"].ap())
        nc.sync.dma_start(out=ptrisb, in_=dram["ptri"].ap())
        w_sliced_dma(nc.sync, wqk, dram["wqk"], 0, 0)            # q pair 0
        w_sliced_dma(nc.sync, wqk, dram["wqk"], C, 8)            # k pair 0
        w_sliced_dma(nc.sync, wqk, dram["wqk"], 128, 1)          # q pair 1
        w_sliced_dma(nc.sync, wqk, dram["wqk"], C + 128, 9)      # k pair 1
        for m in range(2, 8):
            w_sliced_dma(nc.sync, wqk, dram["wqk"], m * 128, m)
            w_sliced_dma(nc.sync, wqk, dram["wqk"], C + m * 128, 8 + m)
        for m in range(8):
            w_sliced_dma(nc.sync, wp, dram["wp"], m * 128, m)

        # scalar (ACT): x and wv — done long before the first exp
        for k2 in range(4):
            nc.scalar.dma_start(
                out=xT.rearrange("p (k t) -> p k t", t=T)[:, 2 * k2:2 * k2 + 2, :],
                in_=dram["xT"].ap().rearrange("(k p) t -> p k t", p=128)
                [:, 2 * k2:2 * k2 + 2, :])
        for k2 in range(2):
            nc.scalar.dma_start(
                out=wv.rearrange("p (k c) -> p k c", c=C)[:, 4 * k2:4 * k2 + 4, :],
                in_=dram["wv"].ap().rearrange("(k p) c -> p k c", p=128)
                [:, 4 * k2:4 * k2 + 4, :])

        def wqk_s(m, k):
            return wqk[:, (m * KC + k) * 128:(m * KC + k) * 128 + 128]

        def wkp_s(m, k):
            return wkp[:, (m * KC + k) * 128:(m * KC + k) * 128 + 128]

        def wp_s(m, k):
            return wp[:, (m * KC + k) * 128:(m * KC + k) * 128 + 128]

        def xT_s(k, sl=None):
            base = xT[:, k * T:(k + 1) * T]
            return base if sl is None else base[:, sl]

        def wv_s(k):
            return wv[:, k * C:(k + 1) * C]

        def wvp_s(k):
            return wvp[:, k * C:(k + 1) * C]

        def pT_s(k):
            return pT[:, k * TP:(k + 1) * TP]

        # zero kpbd once; kpT_group fills the diagonal 64x64 quadrants
        nc.vector.memset(kpbd, 0.0)

        # ---- projection emitters (granular, for filler interleaving) ----
        def kpT_group(m):
            ps = ps_gen.tile([128, TP], F32, tag="ps_g", name="ps_g")
            for k in range(KC):
                pe_chain(nc.tensor.matmul(ps, wkp_s(m, k), pT_s(k),
                                          start=(k == 0), stop=(k == KC - 1)))
            nc.vector.tensor_copy(kpT[m], ps)
            # block-diagonal stationary for the packed prefix-score matmul
            bd = kpbd[:, m * 128:(m + 1) * 128]
            nc.vector.tensor_copy(bd[0:64, 0:64], kpT[m][0:64, :])
            nc.vector.tensor_copy(bd[64:128, 64:128], kpT[m][64:128, :])

        def qk_half(m, hf, _box):
            ps = ps_gen.tile([128, 512], F32, tag="ps_g", name="ps_g")
            for k in range(KC):
                pe_chain(nc.tensor.matmul(
                    ps, wqk_s(m, k), xT_s(k)[:, hf * 512:(hf + 1) * 512],
                    start=(k == 0), stop=(k == KC - 1)))
            nc.vector.tensor_copy(qkT[m][:, hf * 512:(hf + 1) * 512], ps)

        def v_half(tt, hf, _box):
            ps = ps_gen.tile([128, 512], F32, tag="ps_g", name="ps_g")
            sl = slice(tt * 128, (tt + 1) * 128)
            for k in range(KC):
                pe_chain(nc.tensor.matmul(
                    ps, xT_s(k, sl), wv_s(k)[:, hf * 512:(hf + 1) * 512],
                    start=(k == 0), stop=(k == KC - 1)))
            nc.vector.tensor_copy(
                vsb[tt].rearrange("p (h e) -> p h e", e=65)
                [:, hf * 8:(hf + 1) * 8, 0:64],
                ps.rearrange("p (h e) -> p h e", e=64))
            if hf == 1:
                nc.vector.memset(
                    vsb[tt].rearrange("p (h e) -> p h e", e=65)[:, :, 64:65], 1.0)

        def vpsb_group():
            vpv = vpsb.rearrange("p (h e) -> p h e", e=65)
            for hf in range(2):
                ps = ps_gen.tile([64, 512], F32, tag="ps_g", name="ps_g")
                for k in range(KC):
                    pe_chain(nc.tensor.matmul(
                        ps, pT_s(k), wvp_s(k)[:, hf * 512:(hf + 1) * 512],
                        start=(k == 0), stop=(k == KC - 1)))
                nc.vector.tensor_copy(vpv[0:64, hf * 8:(hf + 1) * 8, 0:64],
                                      ps.rearrange("p (h e) -> p h e", e=64))
                nc.vector.tensor_copy(vpv[64:128, hf * 8:(hf + 1) * 8, 0:64],
                                      ps.rearrange("p (h e) -> p h e", e=64))
            nc.vector.memset(vpv[:, :, 64:65], 1.0)

        # outproj hf0: chunk (m, half): half k-ranges (0:4, 4:8); same PSUM
        def outproj_half(hf, m, half, ps_box):
            if half == 0:
                ps_box[0] = ps_gen.tile([128, 512], F32, tag="ps_g", name="ps_g")
            ps = ps_box[0]
            for k in range(4 * half, 4 * half + 4):
                pe_chain(nc.tensor.matmul(
                    ps, wp_s(m, k), yT[k][:, hf * 512:(hf + 1) * 512],
                    start=(k == 0), stop=(k == KC - 1)))
            if half == 1:
                stg = pstg.tile([128, 512], BF, tag="stg", name="stg")
                nc.vector.tensor_copy(stg, ps)
                nc.sync.dma_start(
                    out=dram["out"].ap()[m * 128:(m + 1) * 128,
                                         hf * 512:(hf + 1) * 512],
                    in_=stg)
                ps_box[0] = None

        # outproj hf1 partials: k 0:6 accumulated, evacuated bf16 to part1[m]
        part1_box = [None]

        def outproj1_part(m, half, ps_box):
            if half == 0:
                ps_box[0] = ps_gen.tile([128, 512], F32, tag="ps_g", name="ps_g")
            ps = ps_box[0]
            for k in range(3 * half, 3 * half + 3):
                pe_chain(nc.tensor.matmul(
                    ps, wp_s(m, k), yT[k][:, 512:1024],
                    start=(k == 0), stop=(k == 5)))
            if half == 1:
                part1 = part1_box[0]
                nc.vector.tensor_copy(part1[:, m * 512:(m + 1) * 512], ps)
                ps_box[0] = None

        # ---- filler queue ----
        fillers = []

        def add_group(key, fn, nargs):
            box = [None]
            fillers.append((key, lambda: fn(*nargs, 0, box)))
            fillers.append((key, lambda: fn(*nargs, 1, box)))

        for p in range(2, 8):
            fillers.append(((p, 0), lambda m=p: qk_half(m, 0, None)))
            fillers.append(((p, 0), lambda m=8 + p: qk_half(m, 0, None)))
        for p in range(8):
            fillers.append(((p, 1), lambda m=p: qk_half(m, 1, None)))
            fillers.append(((p, 1), lambda m=8 + p: qk_half(m, 1, None)))
        for m in range(8):
            add_group("op0", outproj_half, (0, m))
        for m in range(8):
            add_group("op1", outproj1_part, (m,))

        def pull_filler(allow_op0, allow_op1=False):
            while fillers:
                key, f = fillers[0]
                if key == "op0" and not allow_op0:
                    return False
                if key == "op1" and not allow_op1:
                    return False
                fillers.pop(0)
                f()
                return True
            return False

        def drain_until(stage_key):
            while any(k == stage_key for k, _ in fillers):
                key, f = fillers.pop(0)
                f()

        class Stage:
            def __init__(self, p, ir):
                self.p, self.ir = p, ir
                self.i0 = ir * 512
                self.jmax = 4 * (ir + 1)
                self.qt, self.kt = qkT[p], qkT[8 + p]
                self.s_all, self.e_all = {}, {}

            def scores(self, jb):
                c0 = max(0, jb - 4 * self.ir) * 128
                st = ps_gen.tile([128, 1024], F32, tag="ps_g", name="ps_g")
                diag = jb >= 4 * self.ir
                for hh, pb in enumerate((0, 64)):
                    pe_chain(nc.tensor.matmul(
                        st[:, hh * 512 + c0:hh * 512 + 512],
                        self.kt[pb:pb + 64, jb * 128:(jb + 1) * 128],
                        self.qt[pb:pb + 64, self.i0 + c0:self.i0 + 512],
                        start=True, stop=not diag))
                if diag:
                    # fold the causal mask in: -300 strict-lower triangle
                    # (key>query) accumulated onto the diagonal 128x128 block
                    for hh in range(2):
                        pe_chain(nc.tensor.matmul(
                            st[:, hh * 512 + c0:hh * 512 + c0 + 128],
                            trisb, identsb,
                            start=False, stop=True, skip_group_check=True))
                self.s_all[jb] = st

            def exps(self, jb):
                c0 = max(0, jb - 4 * self.ir) * 128
                st = self.s_all.pop(jb)
                et = pexp.tile([128, 1024], BF, tag="et", name="et")
                nc.scalar.activation(
                    et.rearrange("p (g n) -> p g n", g=2)[:, :, c0:512],
                    st.rearrange("p (g n) -> p g n", g=2)[:, :, c0:512],
                    Exp, scale=0.125)
                self.e_all[jb] = et

            def avs(self, jb):
                c0 = max(0, jb - 4 * self.ir) * 128
                et = self.e_all.pop(jb)
                for hh in range(2):
                    h = 2 * self.p + hh
                    pe_chain(nc.tensor.matmul(
                        self.Ats[:, hh * 512 + c0:hh * 512 + 512],
                        vsb[jb][:, h * 65:(h + 1) * 65],
                        et[:, hh * 512 + c0:hh * 512 + 512],
                        start=(jb == 0), stop=(jb == self.jmax - 1),
                        skip_group_check=True))

            def front1(self):
                # prefix scores, both heads in one K=128 matmul (block-diag)
                spt = ps_gen.tile([128, 512], F32, tag="ps_g", name="ps_g")
                pe_chain(nc.tensor.matmul(
                    spt, kpbd[:, self.p * 128:(self.p + 1) * 128],
                    self.qt[:, self.i0:self.i0 + 512],
                    start=True, stop=(self.ir == 1)))
                if self.ir == 0:
                    # prefix causal mask on the first 64 query columns
                    pe_chain(nc.tensor.matmul(
                        spt[:, 0:64], ptrisb, identsb[0:64, 0:64],
                        start=False, stop=True, skip_group_check=True))
                self.scores(0)
                ep = pep.tile([128, 512], BF, tag="ep", name="ep")
                nc.scalar.activation(ep, spt, Exp, scale=0.125)
                self.eps = ep
                self.exps(0)

            def front2(self):
                self.scores(1)
                self.Bts = ps_acc.tile([65, 1024], F32, tag="ps_a", name="ps_a")
                for hh, pb in enumerate((0, 64)):
                    h = 2 * self.p + hh
                    pe_chain(nc.tensor.matmul(
                        self.Bts[:, hh * 512:hh * 512 + 512],
                        vpsb[pb:pb + 64, h * 65:(h + 1) * 65],
                        self.eps[pb:pb + 64, :],
                        start=True, stop=True))
                self.exps(1)

            def front2b(self):
                # evacuate B: sums row to recip scratch (ACT, tiny), data to
                # SBUF bf16 (DVE; gpsimd cannot read PSUM). Frees Bts mid-stage.
                self.rs = prs.tile([33, 1024], F32, tag="rs", name="rs")
                nc.scalar.copy(self.rs[0:1, :], self.Bts[64:65, :])
                self.tB = ptb.tile([64, 1024], BF, tag="tB", name="tB")
                nc.vector.tensor_copy(self.tB, self.Bts[0:64, :])
                self.Ats = ps_acc.tile([65, 1024], F32, tag="ps_a", name="ps_a")

            def main(self, allow_op0, allow_op1=False):
                for jb0 in range(0, self.jmax, 2):
                    for jb in (jb0 + 2, jb0 + 3):
                        if jb < self.jmax:
                            self.scores(jb)
                    for jb in (jb0 + 2, jb0 + 3):
                        if jb < self.jmax:
                            self.exps(jb)
                    pull_filler(allow_op0, allow_op1)
                    for jb in (jb0, jb0 + 1):
                        if jb < self.jmax - 1:
                            self.avs(jb)

            def av_last(self):
                self.avs(self.jmax - 1)

            def evacA(self):
                # A sums -> recip scratch row 32 (ACT, tiny); A data -> SBUF
                # bf16 (DVE). This frees the Ats slot: the only thing the next
                # stage's AV waits on is this one DVE copy.
                nc.scalar.copy(self.rs[32:33, :], self.Ats[64:65, :])
                self.tA = pta.tile([64, 1024], BF, tag="tA", name="tA")
                nc.vector.tensor_copy(self.tA, self.Ats[0:64, :])

            def combine(self):
                # recip both sum rows, broadcast per-query recips across
                # partitions with casting gpsimd DMAs, normalize + add.
                # Entirely off the PE.
                rs = self.rs
                nc.vector.reciprocal_approx_fast(rs, rs)
                bcA = pbc.tile([64, 1024], F32, tag="bcA", name="bcA")
                bcB = pbc.tile([64, 1024], F32, tag="bcB", name="bcB")
                rA, rB = rs[32:33, :], rs[0:1, :]
                srcA = bass.AP(tensor=rA.tensor, offset=rA.offset,
                               ap=[[rA.ap[0][0], 1], [0, 64], list(rA.ap[1])])
                srcB = bass.AP(tensor=rB.tensor, offset=rB.offset,
                               ap=[[rB.ap[0][0], 1], [0, 64], list(rB.ap[1])])
                nc.sync.dma_start(out=bcA, in_=srcA)
                nc.sync.dma_start(out=bcB, in_=srcB)
                # muls+adds on gpsimd: they wait on the sync DMAs, and a wait
                # must not block the DVE FIFO (PSUM evacuations live there)
                uA = pua.tile([64, 1024], BF, tag="uA", name="uA")
                nc.gpsimd.tensor_mul(uA, self.tA, bcA)
                uB = pua.tile([64, 1024], BF, tag="uB", name="uB")
                nc.gpsimd.tensor_mul(uB, self.tB, bcB)
                sl = slice(self.i0, self.i0 + 512)
                nc.gpsimd.tensor_add(yT[self.p][0:64, sl],
                                     uA[:, 0:512], uB[:, 0:512])
                nc.gpsimd.tensor_add(yT[self.p][64:128, sl],
                                     uA[:, 512:1024], uB[:, 512:1024])

        # ---- pre-loop: everything that reads pT/wkp/wv/wvp, so their pools
        # can close before the attention pools open (LIFO order) ----
        box = [None]
        for m in range(8):
            kpT_group(m)
        vpsb_group()
        qk_half(0, 0, box); qk_half(8, 0, box)
        qk_half(1, 0, box); qk_half(9, 0, box)
        for tt in range(8):
            v_half(tt, 0, box); v_half(tt, 1, box)
        pwvp.close()   # frees wvp's 16KB
        pearly.close()  # frees pT/wkp/wv (33KB) for part1 + pools below

        plate = top.enter_context(tc.tile_pool(name="plate", bufs=1))
        part1_box[0] = plate.tile([128, 8 * 512], BF, tag="part1", name="part1")
        pexp = top.enter_context(tc.tile_pool(name="pexp", bufs=5))
        pep = top.enter_context(tc.tile_pool(name="pep", bufs=2))
        ptb = top.enter_context(tc.tile_pool(name="ptb", bufs=2))
        pta = top.enter_context(tc.tile_pool(name="pta", bufs=2))
        pbc = top.enter_context(tc.tile_pool(name="pbc", bufs=1))
        pua = top.enter_context(tc.tile_pool(name="pua", bufs=1))
        prs = top.enter_context(tc.tile_pool(name="prs", bufs=2))
        pstg = top.enter_context(tc.tile_pool(name="pstg", bufs=2))
        # recip scratch rotates through two slots; unused rows 1:32 flow
        # through the reciprocal each stage, pin them to 1.0 once.
        for _ in range(2):
            rs_init = prs.tile([33, 1024], F32, tag="rs", name="rs_init")
            nc.vector.memset(rs_init, 1.0)

        # ---- stage loop ----
        stages = [(p, 0) for p in range(8)] + [(p, 1) for p in range(8)]
        prev = None
        for si, (p, ir) in enumerate(stages):
            drain_until((p, ir))
            st = Stage(p, ir)
            st.front1()
            if prev is not None:
                prev.av_last()
                prev.evacA()
            st.front2()
            st.front2b()
            if prev is not None:
                prev.combine()
            st.main(allow_op0=(ir == 1), allow_op1=(si >= 14))
            prev = st
        prev.av_last()
        prev.evacA()
        prev.combine()

        while pull_filler(True, True):
            pass

        # ---- tail: outproj hf1, k 6:8 + bf16 partial add ----
        for m in range(8):
            ps = ps_gen.tile([128, 512], F32, tag="ps_g", name="ps_g")
            for k in (6, 7):
                pe_chain(nc.tensor.matmul(
                    ps, wp_s(m, k), yT[k][:, 512:1024],
                    start=(k == 6), stop=(k == 7)))
            t2 = pstg.tile([128, 512], BF, tag="stg", name="stg")
            nc.vector.tensor_copy(t2, ps)
            stgf = pstg.tile([128, 512], BF, tag="stg", name="stg")
            nc.vector.tensor_add(stgf, t2,
                                 part1_box[0][:, m * 512:(m + 1) * 512])
            nc.sync.dma_start(
                out=dram["out"].ap()[m * 128:(m + 1) * 128, 512:1024],
                in_=stgf)


def _build():
    if "nc" in _CACHE:
        return _CACHE["nc"]
    import concourse.mybir as mybir
    import concourse.tile as tile
    from concourse import bacc

    BF = mybir.dt.bfloat16
    nc = bacc.Bacc("TRN2", target_bir_lowering=False, debug=False,
                   enable_asserts=False)
    dram = {
        "xT": nc.dram_tensor("xT", [C, T], BF, kind="ExternalInput"),
        "pT": nc.dram_tensor("pT", [C, TP], BF, kind="ExternalInput"),
        "wqk": nc.dram_tensor("wqk", [C, 2 * C], BF, kind="ExternalInput"),
        "wv": nc.dram_tensor("wv", [C, C], BF, kind="ExternalInput"),
        "wkp": nc.dram_tensor("wkp", [C, C], BF, kind="ExternalInput"),
        "wvp": nc.dram_tensor("wvp", [C, C], BF, kind="ExternalInput"),
        "wp": nc.dram_tensor("wp", [C, C], BF, kind="ExternalInput"),
        "ident": nc.dram_tensor("ident", [128, 128], BF, kind="ExternalInput"),
        "tri": nc.dram_tensor("tri", [128, 128], BF, kind="ExternalInput"),
        "ptri": nc.dram_tensor("ptri", [64, 128], BF, kind="ExternalInput"),
        "out": nc.dram_tensor("out", [C, T], BF, kind="ExternalOutput"),
    }
    with tile.TileContext(nc) as tc:
        _emit(nc, tc, dram)
    nc.compile()
    _CACHE["nc"] = nc
    return nc


def _host_consts():
    bf = ml_dtypes.bfloat16
    ident = np.eye(128, dtype=np.float32).astype(bf)
    tri = (np.triu(np.ones((128, 128), np.float32), 1) * -300.0).astype(bf)
    p1 = np.triu(np.ones((64, 64), np.float32), 1) * -300.0
    ptri = np.concatenate([p1, p1], axis=1).astype(bf)   # [64 q, 128 keys]
    return ident, tri, ptri


def _make_in_maps(x, prefix_embd, w_attn, w_prefix, w_proj):
    bf = ml_dtypes.bfloat16
    x = np.asarray(x, np.float32)
    prefix_embd = np.asarray(prefix_embd, np.float32)
    w_attn = np.asarray(w_attn, np.float32)
    w_prefix = np.asarray(w_prefix, np.float32)
    w_proj = np.asarray(w_proj, np.float32)
    ident, tri, ptri = _host_consts()
    wqk = np.ascontiguousarray(w_attn[:, :2 * C]).astype(bf)
    wv = np.ascontiguousarray(w_attn[:, 2 * C:]).astype(bf)
    wkp = np.ascontiguousarray(w_prefix[:, C:2 * C]).astype(bf)
    wvp = np.ascontiguousarray(w_prefix[:, 2 * C:]).astype(bf)
    wp = w_proj.astype(bf)
    in_maps = []
    for i in range(B):
        in_maps.append({
            "xT": np.ascontiguousarray(x[i].T).astype(bf),
            "pT": np.ascontiguousarray(prefix_embd[i].T).astype(bf),
            "wqk": wqk, "wv": wv, "wkp": wkp, "wvp": wvp, "wp": wp,
            "ident": ident, "tri": tri, "ptri": ptri,
        })
    return in_maps


def kernel(x, prefix_embd, w_attn, b_attn, w_prefix, b_prefix, w_proj, b_proj,
           **_ignored):
    nc = _build()
    in_maps = _make_in_maps(x, prefix_embd, w_attn, w_prefix, w_proj)
    from concourse.bass_utils import run_bass_kernel_spmd
    res = run_bass_kernel_spmd(nc, in_maps, core_ids=list(range(B)))
    out = np.stack([res.results[i]["out"].T.astype(np.float32)
                    for i in range(B)])
    return np.ascontiguousarray(out)
